# revision 35
# baseline (speedup 1.0000x reference)
"""ArcFace head (B=1024, D=512, C=100000) on 8 TRN2 NeuronCores.

Sharding: tensor-parallel along the num_classes axis (partial-FC ArcFace).
Each core holds a [D, C/8] slice of the (pre-normalized, pre-transposed)
weight and computes its [B, C/8] slice of S * cosine via a bf16 TensorE
matmul with fp32 PSUM accumulation. Embeddings (normalized, scaled by S,
transposed to [D, B]) are broadcast to all cores. The per-row additive
angular margin touches exactly B elements of the [B, C] output, so it is
applied on the host after the gather (exact trig identity:
cos(theta + m) = c*cos(m) - sqrt(1-c^2)*sin(m)).
"""

import os

import numpy as np
import ml_dtypes

import concourse.bass as bass
import concourse.mybir as mybir
from concourse import bacc
from concourse.tile import TileContext
from concourse.bass import ts
from concourse.bass_utils import run_bass_kernel_spmd

# Problem constants (hardcoded per spec)
B, D, C = 1024, 512, 100000
NCORES = 8
CS = C // NCORES          # 12500 classes per core
S, MARGIN, EPS = 30.0, 0.5, 1e-7

P = 128                   # partitions
KS = D // P               # 4 k-subtiles
MS = B // P               # 8 m-subtiles
NT = 512                  # n tile (one PSUM bank of fp32)
# the device computes the largest NT-aligned prefix of each core's CS columns;
# the ragged remainder (212 columns/core, 1.7% of the FLOPs) is computed on
# the host in fp32 — it would otherwise cost inefficient 424B-descriptor DMAs
# and a partial-width matmul pass
DEV_CS = (CS // NT) * NT  # 12288
REM = CS - DEV_CS         # 212

BF16 = mybir.dt.bfloat16
_bf16_np = ml_dtypes.bfloat16


def build_nc():
    nc = bacc.Bacc(None, target_bir_lowering=False)
    embT = nc.dram_tensor("embT", [D, B], BF16, kind="ExternalInput")
    wT = nc.dram_tensor("wT", [D, DEV_CS], BF16, kind="ExternalInput")
    out = nc.dram_tensor("out", [B, DEV_CS], BF16, kind="ExternalOutput")

    WARMUP_MMS = 62

    with TileContext(nc) as tc:
        with (
            tc.tile_pool(name="emb", bufs=1) as epool,
            tc.tile_pool(name="w", bufs=3) as wpool,
            tc.tile_pool(name="o", bufs=4) as opool,
            tc.tile_pool(name="ps", bufs=8, space="PSUM") as pspool,
        ):
            embT_r = embT[:].rearrange("(ko p) b -> p ko b", p=P)
            wT_r = wT[:].rearrange("(ko p) c -> p ko c", p=P)
            out_r = out[:].rearrange("(mo p) c -> p mo c", p=P)

            # PE warm-up: dummy matmuls run during the initial DMA wait so the
            # HAM clock gate is at 2.4 GHz when the real MM stream starts.
            dummy = epool.tile([P, 64], BF16, tag="dummy")
            nc.vector.memset(dummy[:], 0.0)
            wps = pspool.tile([P, NT], mybir.dt.float32, tag="ps")
            for _ in range(WARMUP_MMS):
                nc.tensor.matmul(
                    wps[:64, :64], lhsT=dummy[:64, :], rhs=dummy[:64, :],
                    start=True, stop=True,
                )

            # per-k embedding tiles so the first matmul waits on 256KB, not
            # 1MB; interleave their DMA issues with the first weight tile's
            # per-k chunks so the k=0 matmuls' data (w_k0 + emb_k0) lands
            # first on the ring
            emb_sb = [
                epool.tile([P, B], BF16, tag=f"emb{k}", name=f"emb_{k}")
                for k in range(KS)
            ]
            w_first = wpool.tile([P, KS, 2 * NT], BF16, tag="w", name="w_first")
            # the very first matmul needs only w[k0, :512] + emb[k0]: issue
            # those two smallest chunks first so its data lands earliest
            nc.sync.dma_start(out=w_first[:, 0, :NT], in_=wT_r[:, 0, 0:NT])
            nc.sync.dma_start(out=emb_sb[0][:], in_=embT_r[:, 0, :])
            nc.sync.dma_start(out=w_first[:, 0, NT : 2 * NT], in_=wT_r[:, 0, NT : 2 * NT])
            for k in range(1, KS):
                nc.sync.dma_start(out=w_first[:, k, :], in_=wT_r[:, k, 0 : 2 * NT])
                nc.sync.dma_start(out=emb_sb[k][:], in_=embT_r[:, k, :])

            # super-tiles of 1024 columns -> 2KB DMA descriptors (vs 1KB at
            # 512): roughly halves DMA engine occupancy for the same bytes.
            # Order: one full tile first (chunked per-k so matmuls start on
            # partial data), two narrow 512 tiles last so the kernel tail
            # flushes a small final transfer.
            supers = (
                [(i * 2 * NT, 2 * NT) for i in range(11)]
                + [(11 * 2 * NT, NT), (11 * 2 * NT + NT, NT)]
            )
            first = True
            for idx, (n0, nw) in enumerate(supers):
                last_tile = idx == len(supers) - 1
                if first:
                    w_sb = w_first
                    first = False
                else:
                    w_sb = wpool.tile([P, KS, 2 * NT], BF16, tag="w", name=f"w_{n0}")
                    nc.sync.dma_start(
                        out=w_sb[:, :, :nw], in_=wT_r[:, :, n0 : n0 + nw]
                    )
                o_sb = opool.tile([P, MS, 2 * NT], BF16, tag="o")
                for h in range(2):
                    h0 = h * NT
                    hw = min(NT, nw - h0)
                    if hw <= 0:
                        continue
                    for m in range(MS):
                        ps = pspool.tile(
                            [P, NT], mybir.dt.float32, tag="ps", name=f"ps_{n0}_{h}_{m}"
                        )
                        for k in range(KS):
                            nc.tensor.matmul(
                                ps[:, :hw],
                                lhsT=emb_sb[k][:, ts(m, P)],
                                rhs=w_sb[:, k, h0 : h0 + hw],
                                start=(k == 0),
                                stop=(k == KS - 1),
                            )
                        # split PSUM->SBUF cast copies between ACT and DVE
                        if m % 2 == 0:
                            nc.scalar.copy(
                                out=o_sb[:, m, h0 : h0 + hw], in_=ps[:, :hw]
                            )
                        else:
                            nc.vector.tensor_copy(
                                out=o_sb[:, m, h0 : h0 + hw], in_=ps[:, :hw]
                            )
                        # half-tile output DMAs (by m-range, keeping rows
                        # contiguous): second half streams out while the next
                        # tile computes; keeps the kernel tail short. The very
                        # last tile flushes in m-pair quarters so the final
                        # transfer (the one the exit barrier waits on) is tiny.
                        last_h = (h == 1) or (nw <= NT)
                        if last_h and last_tile and m % 2 == 1:
                            nc.sync.dma_start(
                                out=out_r[:, m - 1 : m + 1, n0 : n0 + nw],
                                in_=o_sb[:, m - 1 : m + 1, :nw],
                            )
                        elif not last_tile and last_h and m == MS // 2 - 1:
                            nc.sync.dma_start(
                                out=out_r[:, 0 : MS // 2, n0 : n0 + nw],
                                in_=o_sb[:, 0 : MS // 2, :nw],
                            )
                        elif not last_tile and last_h and m == MS - 1:
                            nc.sync.dma_start(
                                out=out_r[:, MS // 2 : MS, n0 : n0 + nw],
                                in_=o_sb[:, MS // 2 : MS, :nw],
                            )
    nc.finalize()
    return nc


_NC_CACHE = []


def _get_nc():
    if not _NC_CACHE:
        _NC_CACHE.append(build_nc())
    return _NC_CACHE[0]


def _prep_in_maps(embeddings, weight):
    # normalize on host (fp32), fold the ArcFace scale S into the embeddings
    en = embeddings / np.maximum(
        np.linalg.norm(embeddings, axis=1, keepdims=True), 1e-12
    )
    wn = weight / np.maximum(np.linalg.norm(weight, axis=1, keepdims=True), 1e-12)
    embT = np.ascontiguousarray((S * en).T).astype(_bf16_np)  # [D, B]
    wTn = wn.T  # [D, C] view
    in_maps = []
    for i in range(NCORES):
        shard = np.ascontiguousarray(
            wTn[:, i * CS : i * CS + DEV_CS]
        ).astype(_bf16_np)
        in_maps.append({"embT": embT, "wT": shard})
    return in_maps, en, wn


def run_device(embeddings, weight, **spmd_kwargs):
    """Runs the device part; returns (full S*cosine [B, C] fp32, raw results)."""
    if not spmd_kwargs.get("trace"):
        # the axon NTFF-profile hook may be absent in this image; make sure an
        # ambient BASS_TRACE env var can't route us onto that path
        os.environ.setdefault("BASS_NEVER_TRACE", "1")
    nc = _get_nc()
    in_maps, en, wn = _prep_in_maps(embeddings, weight)
    try:
        res = run_bass_kernel_spmd(
            nc, in_maps, core_ids=list(range(NCORES)), **spmd_kwargs
        )
    except Exception:
        # rare transient NRT_EXEC_UNIT_UNRECOVERABLE faults have been observed
        # on this fleet (~2 in 12 runs, uncorrelated with kernel structure);
        # one retry costs nothing if the fault persists
        res = run_bass_kernel_spmd(
            nc, in_maps, core_ids=list(range(NCORES)), **spmd_kwargs
        )
    # ragged remainder columns (212 per core) in fp32 on the host
    rem_w = np.concatenate(
        [wn[i * CS + DEV_CS : (i + 1) * CS] for i in range(NCORES)], axis=0
    )  # [NCORES*REM, D]
    rem_out = (S * en) @ rem_w.T  # [B, NCORES*REM]
    out = np.empty((B, C), dtype=np.float32)
    for i in range(NCORES):
        out[:, i * CS : i * CS + DEV_CS] = np.asarray(
            res.results[i]["out"]
        ).astype(np.float32)
        out[:, i * CS + DEV_CS : (i + 1) * CS] = rem_out[
            :, i * REM : (i + 1) * REM
        ]
    return out, res


def apply_margin(out, labels):
    rows = np.arange(B)
    lab = np.asarray(labels).astype(np.int64)
    c = np.clip(out[rows, lab] / S, -1.0 + EPS, 1.0 - EPS)
    out[rows, lab] = S * (c * np.cos(MARGIN) - np.sqrt(1.0 - c * c) * np.sin(MARGIN))
    return out


def kernel(embeddings, weight, labels):
    embeddings = np.asarray(embeddings, dtype=np.float32)
    weight = np.asarray(weight, dtype=np.float32)
    out, _ = run_device(embeddings, weight)
    return apply_margin(out, labels)


# revision 36
# speedup vs baseline: 1.0002x; 1.0002x over previous
"""ArcFace head (B=1024, D=512, C=100000) on 8 TRN2 NeuronCores.

Sharding: tensor-parallel along the num_classes axis (partial-FC ArcFace).
Each core holds a [D, C/8] slice of the (pre-normalized, pre-transposed)
weight and computes its [B, C/8] slice of S * cosine via a bf16 TensorE
matmul with fp32 PSUM accumulation. Embeddings (normalized, scaled by S,
transposed to [D, B]) are broadcast to all cores. The per-row additive
angular margin touches exactly B elements of the [B, C] output, so it is
applied on the host after the gather (exact trig identity:
cos(theta + m) = c*cos(m) - sqrt(1-c^2)*sin(m)).
"""

import os

import numpy as np
import ml_dtypes

import concourse.bass as bass
import concourse.mybir as mybir
from concourse import bacc
from concourse.tile import TileContext
from concourse.bass import ts
from concourse.bass_utils import run_bass_kernel_spmd

# Problem constants (hardcoded per spec)
B, D, C = 1024, 512, 100000
NCORES = 8
CS = C // NCORES          # 12500 classes per core
S, MARGIN, EPS = 30.0, 0.5, 1e-7

P = 128                   # partitions
KS = D // P               # 4 k-subtiles
MS = B // P               # 8 m-subtiles
NT = 512                  # n tile (one PSUM bank of fp32)
# the device computes the largest NT-aligned prefix of each core's CS columns;
# the ragged remainder (212 columns/core, 1.7% of the FLOPs) is computed on
# the host in fp32 — it would otherwise cost inefficient 424B-descriptor DMAs
# and a partial-width matmul pass
DEV_CS = (CS // NT) * NT  # 12288
REM = CS - DEV_CS         # 212

BF16 = mybir.dt.bfloat16
_bf16_np = ml_dtypes.bfloat16


def build_nc():
    nc = bacc.Bacc(None, target_bir_lowering=False)
    embT = nc.dram_tensor("embT", [D, B], BF16, kind="ExternalInput")
    wT = nc.dram_tensor("wT", [D, DEV_CS], BF16, kind="ExternalInput")
    out = nc.dram_tensor("out", [B, DEV_CS], BF16, kind="ExternalOutput")

    WARMUP_MMS = 62

    with TileContext(nc) as tc:
        with (
            tc.tile_pool(name="emb", bufs=1) as epool,
            tc.tile_pool(name="w", bufs=3) as wpool,
            tc.tile_pool(name="o", bufs=4) as opool,
            tc.tile_pool(name="ps", bufs=8, space="PSUM") as pspool,
        ):
            embT_r = embT[:].rearrange("(ko p) b -> p ko b", p=P)
            wT_r = wT[:].rearrange("(ko p) c -> p ko c", p=P)
            out_r = out[:].rearrange("(mo p) c -> p mo c", p=P)

            # PE warm-up: dummy matmuls run during the initial DMA wait so the
            # HAM clock gate is at 2.4 GHz when the real MM stream starts.
            dummy = epool.tile([P, 64], BF16, tag="dummy")
            nc.vector.memset(dummy[:], 0.0)
            wps = pspool.tile([P, NT], mybir.dt.float32, tag="ps")
            for _ in range(WARMUP_MMS):
                nc.tensor.matmul(
                    wps[:64, :64], lhsT=dummy[:64, :], rhs=dummy[:64, :],
                    start=True, stop=True,
                )

            # per-k embedding tiles so the first matmul waits on 256KB, not
            # 1MB; interleave their DMA issues with the first weight tile's
            # per-k chunks so the k=0 matmuls' data (w_k0 + emb_k0) lands
            # first on the ring
            emb_sb = [
                epool.tile([P, B], BF16, tag=f"emb{k}", name=f"emb_{k}")
                for k in range(KS)
            ]
            w_first = wpool.tile([P, KS, 2 * NT], BF16, tag="w", name="w_first")
            # the very first matmul needs only w[k0, :512] + emb[k0]: issue
            # those two smallest chunks first so its data lands earliest; k1
            # (needed by the second matmul of the group) goes next, and the
            # k0 second-half (not needed until h=1) is deferred to the end
            nc.sync.dma_start(out=w_first[:, 0, :NT], in_=wT_r[:, 0, 0:NT])
            nc.sync.dma_start(out=emb_sb[0][:], in_=embT_r[:, 0, :])
            for k in range(1, KS):
                nc.sync.dma_start(out=w_first[:, k, :], in_=wT_r[:, k, 0 : 2 * NT])
                nc.sync.dma_start(out=emb_sb[k][:], in_=embT_r[:, k, :])
            nc.sync.dma_start(out=w_first[:, 0, NT : 2 * NT], in_=wT_r[:, 0, NT : 2 * NT])

            # super-tiles of 1024 columns -> 2KB DMA descriptors (vs 1KB at
            # 512): roughly halves DMA engine occupancy for the same bytes.
            # Order: one full tile first (chunked per-k so matmuls start on
            # partial data), two narrow 512 tiles last so the kernel tail
            # flushes a small final transfer.
            supers = (
                [(i * 2 * NT, 2 * NT) for i in range(11)]
                + [(11 * 2 * NT, NT), (11 * 2 * NT + NT, NT)]
            )
            first = True
            for idx, (n0, nw) in enumerate(supers):
                last_tile = idx == len(supers) - 1
                if first:
                    w_sb = w_first
                    first = False
                else:
                    w_sb = wpool.tile([P, KS, 2 * NT], BF16, tag="w", name=f"w_{n0}")
                    nc.sync.dma_start(
                        out=w_sb[:, :, :nw], in_=wT_r[:, :, n0 : n0 + nw]
                    )
                o_sb = opool.tile([P, MS, 2 * NT], BF16, tag="o")
                for h in range(2):
                    h0 = h * NT
                    hw = min(NT, nw - h0)
                    if hw <= 0:
                        continue
                    for m in range(MS):
                        ps = pspool.tile(
                            [P, NT], mybir.dt.float32, tag="ps", name=f"ps_{n0}_{h}_{m}"
                        )
                        for k in range(KS):
                            nc.tensor.matmul(
                                ps[:, :hw],
                                lhsT=emb_sb[k][:, ts(m, P)],
                                rhs=w_sb[:, k, h0 : h0 + hw],
                                start=(k == 0),
                                stop=(k == KS - 1),
                            )
                        # split PSUM->SBUF cast copies between ACT and DVE
                        if m % 2 == 0:
                            nc.scalar.copy(
                                out=o_sb[:, m, h0 : h0 + hw], in_=ps[:, :hw]
                            )
                        else:
                            nc.vector.tensor_copy(
                                out=o_sb[:, m, h0 : h0 + hw], in_=ps[:, :hw]
                            )
                        # half-tile output DMAs (by m-range, keeping rows
                        # contiguous): second half streams out while the next
                        # tile computes; keeps the kernel tail short. The very
                        # last tile flushes in m-pair quarters so the final
                        # transfer (the one the exit barrier waits on) is tiny.
                        last_h = (h == 1) or (nw <= NT)
                        if last_h and last_tile and m % 2 == 1:
                            nc.sync.dma_start(
                                out=out_r[:, m - 1 : m + 1, n0 : n0 + nw],
                                in_=o_sb[:, m - 1 : m + 1, :nw],
                            )
                        elif not last_tile and last_h and m == MS // 2 - 1:
                            nc.sync.dma_start(
                                out=out_r[:, 0 : MS // 2, n0 : n0 + nw],
                                in_=o_sb[:, 0 : MS // 2, :nw],
                            )
                        elif not last_tile and last_h and m == MS - 1:
                            nc.sync.dma_start(
                                out=out_r[:, MS // 2 : MS, n0 : n0 + nw],
                                in_=o_sb[:, MS // 2 : MS, :nw],
                            )
    nc.finalize()
    return nc


_NC_CACHE = []


def _get_nc():
    if not _NC_CACHE:
        _NC_CACHE.append(build_nc())
    return _NC_CACHE[0]


def _prep_in_maps(embeddings, weight):
    # normalize on host (fp32), fold the ArcFace scale S into the embeddings
    en = embeddings / np.maximum(
        np.linalg.norm(embeddings, axis=1, keepdims=True), 1e-12
    )
    wn = weight / np.maximum(np.linalg.norm(weight, axis=1, keepdims=True), 1e-12)
    embT = np.ascontiguousarray((S * en).T).astype(_bf16_np)  # [D, B]
    wTn = wn.T  # [D, C] view
    in_maps = []
    for i in range(NCORES):
        shard = np.ascontiguousarray(
            wTn[:, i * CS : i * CS + DEV_CS]
        ).astype(_bf16_np)
        in_maps.append({"embT": embT, "wT": shard})
    return in_maps, en, wn


def run_device(embeddings, weight, **spmd_kwargs):
    """Runs the device part; returns (full S*cosine [B, C] fp32, raw results)."""
    if not spmd_kwargs.get("trace"):
        # the axon NTFF-profile hook may be absent in this image; make sure an
        # ambient BASS_TRACE env var can't route us onto that path
        os.environ.setdefault("BASS_NEVER_TRACE", "1")
    nc = _get_nc()
    in_maps, en, wn = _prep_in_maps(embeddings, weight)
    try:
        res = run_bass_kernel_spmd(
            nc, in_maps, core_ids=list(range(NCORES)), **spmd_kwargs
        )
    except Exception:
        # rare transient NRT_EXEC_UNIT_UNRECOVERABLE faults have been observed
        # on this fleet (~2 in 12 runs, uncorrelated with kernel structure);
        # one retry costs nothing if the fault persists
        res = run_bass_kernel_spmd(
            nc, in_maps, core_ids=list(range(NCORES)), **spmd_kwargs
        )
    # ragged remainder columns (212 per core) in fp32 on the host
    rem_w = np.concatenate(
        [wn[i * CS + DEV_CS : (i + 1) * CS] for i in range(NCORES)], axis=0
    )  # [NCORES*REM, D]
    rem_out = (S * en) @ rem_w.T  # [B, NCORES*REM]
    out = np.empty((B, C), dtype=np.float32)
    for i in range(NCORES):
        out[:, i * CS : i * CS + DEV_CS] = np.asarray(
            res.results[i]["out"]
        ).astype(np.float32)
        out[:, i * CS + DEV_CS : (i + 1) * CS] = rem_out[
            :, i * REM : (i + 1) * REM
        ]
    return out, res


def apply_margin(out, labels):
    rows = np.arange(B)
    lab = np.asarray(labels).astype(np.int64)
    c = np.clip(out[rows, lab] / S, -1.0 + EPS, 1.0 - EPS)
    out[rows, lab] = S * (c * np.cos(MARGIN) - np.sqrt(1.0 - c * c) * np.sin(MARGIN))
    return out


def kernel(embeddings, weight, labels):
    embeddings = np.asarray(embeddings, dtype=np.float32)
    weight = np.asarray(weight, dtype=np.float32)
    out, _ = run_device(embeddings, weight)
    return apply_margin(out, labels)


# revision 38
# speedup vs baseline: 1.1770x; 1.1768x over previous
"""ArcFace head (B=1024, D=512, C=100000) on 8 TRN2 NeuronCores.

Sharding: tensor-parallel along the num_classes axis (partial-FC ArcFace).
Each core holds a [D, C/8] slice of the (pre-normalized, pre-transposed)
weight and computes its [B, C/8] slice of S * cosine via a bf16 TensorE
matmul with fp32 PSUM accumulation. Embeddings (normalized, scaled by S,
transposed to [D, B]) are broadcast to all cores. The per-row additive
angular margin touches exactly B elements of the [B, C] output, so it is
applied on the host after the gather (exact trig identity:
cos(theta + m) = c*cos(m) - sqrt(1-c^2)*sin(m)).
"""

import os

import numpy as np
import ml_dtypes

import concourse.bass as bass
import concourse.mybir as mybir
from concourse import bacc
from concourse.tile import TileContext
from concourse.bass import ts
from concourse.bass_utils import run_bass_kernel_spmd

# Problem constants (hardcoded per spec)
B, D, C = 1024, 512, 100000
NCORES = 8
CS = C // NCORES          # 12500 classes per core
S, MARGIN, EPS = 30.0, 0.5, 1e-7

P = 128                   # partitions
KS = D // P               # 4 k-subtiles
MS = B // P               # 8 m-subtiles
NT = 512                  # n tile (one PSUM bank of fp32)
# the device computes the largest NT-aligned prefix of each core's CS columns;
# the ragged remainder (212 columns/core, 1.7% of the FLOPs) is computed on
# the host in fp32 — it would otherwise cost inefficient 424B-descriptor DMAs
# and a partial-width matmul pass
DEV_CS = (CS // NT) * NT  # 12288
REM = CS - DEV_CS         # 212

BF16 = mybir.dt.bfloat16
_bf16_np = ml_dtypes.bfloat16


def build_nc():
    nc = bacc.Bacc(None, target_bir_lowering=False)
    embT = nc.dram_tensor("embT", [D, B], BF16, kind="ExternalInput")
    wT = nc.dram_tensor("wT", [D, DEV_CS], BF16, kind="ExternalInput")
    out = nc.dram_tensor("out", [B, DEV_CS], BF16, kind="ExternalOutput")

    WARMUP_MMS = 62

    with TileContext(nc) as tc:
        with (
            tc.tile_pool(name="emb", bufs=1) as epool,
            tc.tile_pool(name="w", bufs=4) as wpool,
            tc.tile_pool(name="o", bufs=5) as opool,
            tc.tile_pool(name="ps", bufs=8, space="PSUM") as pspool,
        ):
            embT_r = embT[:].rearrange("(ko p) b -> p ko b", p=P)
            wT_r = wT[:].rearrange("(ko p) c -> p ko c", p=P)
            out_r = out[:].rearrange("(mo p) c -> p mo c", p=P)

            # PE warm-up: dummy matmuls run during the initial DMA wait so the
            # HAM clock gate is at 2.4 GHz when the real MM stream starts.
            dummy = epool.tile([P, 64], BF16, tag="dummy")
            nc.vector.memset(dummy[:], 0.0)
            wps = pspool.tile([P, NT], mybir.dt.float32, tag="ps")
            for _ in range(WARMUP_MMS):
                nc.tensor.matmul(
                    wps[:64, :64], lhsT=dummy[:64, :], rhs=dummy[:64, :],
                    start=True, stop=True,
                )

            # per-k embedding tiles so the first matmul waits on 256KB, not
            # 1MB; interleave their DMA issues with the first weight tile's
            # per-k chunks so the k=0 matmuls' data (w_k0 + emb_k0) lands
            # first on the ring
            emb_sb = [
                epool.tile([P, B], BF16, tag=f"emb{k}", name=f"emb_{k}")
                for k in range(KS)
            ]
            w_first = wpool.tile([P, KS, 2 * NT], BF16, tag="w", name="w_first")
            # the very first matmul needs only w[k0, :512] + emb[k0]: issue
            # those two smallest chunks first so its data lands earliest; k1
            # (needed by the second matmul of the group) goes next, and the
            # k0 second-half (not needed until h=1) is deferred to the end
            nc.sync.dma_start(out=w_first[:, 0, :NT], in_=wT_r[:, 0, 0:NT])
            nc.sync.dma_start(out=emb_sb[0][:], in_=embT_r[:, 0, :])
            nc.sync.dma_start(out=w_first[:, 1, :NT], in_=wT_r[:, 1, 0:NT])
            nc.sync.dma_start(out=emb_sb[1][:], in_=embT_r[:, 1, :])
            for k in range(2, KS):
                nc.sync.dma_start(out=w_first[:, k, :], in_=wT_r[:, k, 0 : 2 * NT])
                nc.sync.dma_start(out=emb_sb[k][:], in_=embT_r[:, k, :])
            nc.sync.dma_start(out=w_first[:, 1, NT : 2 * NT], in_=wT_r[:, 1, NT : 2 * NT])
            nc.sync.dma_start(out=w_first[:, 0, NT : 2 * NT], in_=wT_r[:, 0, NT : 2 * NT])

            # super-tiles of 1024 columns -> 2KB DMA descriptors (vs 1KB at
            # 512): roughly halves DMA engine occupancy for the same bytes.
            # Order: one full tile first (chunked per-k so matmuls start on
            # partial data), two narrow 512 tiles last so the kernel tail
            # flushes a small final transfer.
            supers = (
                [(i * 2 * NT, 2 * NT) for i in range(11)]
                + [(11 * 2 * NT, NT), (11 * 2 * NT + NT, NT)]
            )
            first = True
            for idx, (n0, nw) in enumerate(supers):
                last_tile = idx == len(supers) - 1
                if first:
                    w_sb = w_first
                    first = False
                else:
                    w_sb = wpool.tile([P, KS, 2 * NT], BF16, tag="w", name=f"w_{n0}")
                    nc.sync.dma_start(
                        out=w_sb[:, :, :nw], in_=wT_r[:, :, n0 : n0 + nw]
                    )
                o_sb = opool.tile([P, MS, 2 * NT], BF16, tag="o")
                for h in range(2):
                    h0 = h * NT
                    hw = min(NT, nw - h0)
                    if hw <= 0:
                        continue
                    for m in range(MS):
                        ps = pspool.tile(
                            [P, NT], mybir.dt.float32, tag="ps", name=f"ps_{n0}_{h}_{m}"
                        )
                        for k in range(KS):
                            nc.tensor.matmul(
                                ps[:, :hw],
                                lhsT=emb_sb[k][:, ts(m, P)],
                                rhs=w_sb[:, k, h0 : h0 + hw],
                                start=(k == 0),
                                stop=(k == KS - 1),
                            )
                        # split PSUM->SBUF cast copies between ACT and DVE
                        if m % 2 == 0:
                            nc.scalar.copy(
                                out=o_sb[:, m, h0 : h0 + hw], in_=ps[:, :hw]
                            )
                        else:
                            nc.vector.tensor_copy(
                                out=o_sb[:, m, h0 : h0 + hw], in_=ps[:, :hw]
                            )
                        # half-tile output DMAs (by m-range, keeping rows
                        # contiguous): second half streams out while the next
                        # tile computes; keeps the kernel tail short. The very
                        # last tile flushes in m-pair quarters so the final
                        # transfer (the one the exit barrier waits on) is tiny.
                        last_h = (h == 1) or (nw <= NT)
                        if last_h and last_tile and m % 2 == 1:
                            nc.sync.dma_start(
                                out=out_r[:, m - 1 : m + 1, n0 : n0 + nw],
                                in_=o_sb[:, m - 1 : m + 1, :nw],
                            )
                        elif not last_tile and last_h and m == MS // 2 - 1:
                            nc.sync.dma_start(
                                out=out_r[:, 0 : MS // 2, n0 : n0 + nw],
                                in_=o_sb[:, 0 : MS // 2, :nw],
                            )
                        elif not last_tile and last_h and m == MS - 1:
                            nc.sync.dma_start(
                                out=out_r[:, MS // 2 : MS, n0 : n0 + nw],
                                in_=o_sb[:, MS // 2 : MS, :nw],
                            )
    nc.finalize()
    return nc


_NC_CACHE = []


def _get_nc():
    if not _NC_CACHE:
        _NC_CACHE.append(build_nc())
    return _NC_CACHE[0]


def _prep_in_maps(embeddings, weight):
    # normalize on host (fp32), fold the ArcFace scale S into the embeddings
    en = embeddings / np.maximum(
        np.linalg.norm(embeddings, axis=1, keepdims=True), 1e-12
    )
    wn = weight / np.maximum(np.linalg.norm(weight, axis=1, keepdims=True), 1e-12)
    embT = np.ascontiguousarray((S * en).T).astype(_bf16_np)  # [D, B]
    wTn = wn.T  # [D, C] view
    in_maps = []
    for i in range(NCORES):
        shard = np.ascontiguousarray(
            wTn[:, i * CS : i * CS + DEV_CS]
        ).astype(_bf16_np)
        in_maps.append({"embT": embT, "wT": shard})
    return in_maps, en, wn


def run_device(embeddings, weight, **spmd_kwargs):
    """Runs the device part; returns (full S*cosine [B, C] fp32, raw results)."""
    if not spmd_kwargs.get("trace"):
        # the axon NTFF-profile hook may be absent in this image; make sure an
        # ambient BASS_TRACE env var can't route us onto that path
        os.environ.setdefault("BASS_NEVER_TRACE", "1")
    nc = _get_nc()
    in_maps, en, wn = _prep_in_maps(embeddings, weight)
    try:
        res = run_bass_kernel_spmd(
            nc, in_maps, core_ids=list(range(NCORES)), **spmd_kwargs
        )
    except Exception:
        # rare transient NRT_EXEC_UNIT_UNRECOVERABLE faults have been observed
        # on this fleet (~2 in 12 runs, uncorrelated with kernel structure);
        # one retry costs nothing if the fault persists
        res = run_bass_kernel_spmd(
            nc, in_maps, core_ids=list(range(NCORES)), **spmd_kwargs
        )
    # ragged remainder columns (212 per core) in fp32 on the host
    rem_w = np.concatenate(
        [wn[i * CS + DEV_CS : (i + 1) * CS] for i in range(NCORES)], axis=0
    )  # [NCORES*REM, D]
    rem_out = (S * en) @ rem_w.T  # [B, NCORES*REM]
    out = np.empty((B, C), dtype=np.float32)
    for i in range(NCORES):
        out[:, i * CS : i * CS + DEV_CS] = np.asarray(
            res.results[i]["out"]
        ).astype(np.float32)
        out[:, i * CS + DEV_CS : (i + 1) * CS] = rem_out[
            :, i * REM : (i + 1) * REM
        ]
    return out, res


def apply_margin(out, labels):
    rows = np.arange(B)
    lab = np.asarray(labels).astype(np.int64)
    c = np.clip(out[rows, lab] / S, -1.0 + EPS, 1.0 - EPS)
    out[rows, lab] = S * (c * np.cos(MARGIN) - np.sqrt(1.0 - c * c) * np.sin(MARGIN))
    return out


def kernel(embeddings, weight, labels):
    embeddings = np.asarray(embeddings, dtype=np.float32)
    weight = np.asarray(weight, dtype=np.float32)
    out, _ = run_device(embeddings, weight)
    return apply_margin(out, labels)
